# revision 39
# baseline (speedup 1.0000x reference)
"""DetectionLoss Trainium2 kernel.

Strategy (data-parallel over batch): 16 images -> 8 cores x 2 images.
Each core computes, per image: per-anchor "last gt box wins" assignment,
focal-cls / IoU-box / objectness-BCE partial sums; host combines scalars.

Per-anchor assignment trick: anchors form a 256x256 grid; box j covers an
axis-aligned rectangle = outer(row_mask_j, col_mask_j). A PE matmul with
weights 2^(j mod 16) produces, per anchor, S_lo/S_hi = sums of distinct
powers of two over containing boxes (exact in fp32, <= 2^16-1). The IEEE
exponent field of S (bitwise AND + bitcast) yields P = 2^floor(log2(S)),
i.e. the LAST containing box index encoded as an exact power of two:
Q = max(65536 * P_hi, P_lo) = 2^j_last, or 0 if no box contains the anchor.

The winning class logit is gathered with gpsimd ap_gather (indices are
host-provided data = x*80 + gt_label[j], shared across partitions), then
selected by multiplying with the onehot O = (Q == 2^j) and group-reducing.
Assigned-box coords come from the same O against per-box fp16 tables.
"""

import os
import sys
from contextlib import ExitStack

import numpy as np

try:
    import concourse.bass as bass  # noqa: F401
except ImportError:  # pragma: no cover
    sys.path.insert(0, "/opt/trn_rl_repo")
    import concourse.bass as bass

import concourse.bacc as bacc
import concourse.tile as tile
from concourse import mybir
from concourse.bass_utils import run_bass_kernel_spmd

F32 = mybir.dt.float32
BF16 = mybir.dt.bfloat16
F16 = mybir.dt.float16
I32 = mybir.dt.int32
I16 = mybir.dt.int16
ALU = mybir.AluOpType
ACTF = mybir.ActivationFunctionType
AXL = mybir.AxisListType

NUM_CLASSES = 80
ALPHA, GAMMA = 0.25, 2.0
W_CLS, W_BOX, W_OBJ = 1.0, 5.0, 1.0
EPS = 1e-8

EXP_MASK = 0x7F800000  # fp32 exponent field


def _kernel_body(ctx, tc, aps, G, C, M, IPC, XBS, stage=99):
    nc = tc.nc
    P = min(G, 128)
    H = G // P
    NXB = G // XBS
    NI = XBS * M

    cls_v = aps["cls"].rearrange("i (y x) c -> i y x c", x=G)
    box_v = aps["boxes"].rearrange("i (y x) c -> i y x c", x=G)
    obj_v = aps["obj"].rearrange("i (y x) -> i y x", x=G)

    singles = ctx.enter_context(tc.tile_pool(name="singles", bufs=1))
    cpool = ctx.enter_context(tc.tile_pool(name="cls", bufs=2))
    epool = ctx.enter_context(tc.tile_pool(name="exp", bufs=2))
    ppool = ctx.enter_context(tc.tile_pool(name="proj", bufs=2))
    opool = ctx.enter_context(tc.tile_pool(name="oh", bufs=2))
    hpool = ctx.enter_context(tc.tile_pool(name="halfmaps", bufs=2))
    tpool = ctx.enter_context(tc.tile_pool(name="tmp", bufs=1))
    psums = ctx.enter_context(tc.tile_pool(name="psum", bufs=1, space="PSUM"))

    # --- constants / small inputs ---
    # lhsT (row masks * 2^j) and rhs (col masks, lo|hi) combined in one
    # tensor/DMA so each matmul carries at most one sync wait.
    asg_sb = singles.tile([M, IPC, 3 * G], F32)
    nc.default_dma_engine.dma_start(
        out=asg_sb[:], in_=aps["asg"].rearrange("i m g -> m i g")
    )
    pow2_sb = singles.tile([P, M], BF16)
    nc.default_dma_engine.dma_start(out=pow2_sb[:], in_=aps["pow2"][:P])
    tab_sb = singles.tile([P, IPC, 4, M], F16)
    nc.default_dma_engine.dma_start(
        out=tab_sb[:], in_=aps["tab16"].rearrange("i p c m -> p i c m")[:P]
    )
    gidx_sb = singles.tile([P, IPC, NI // 16], I16)
    nc.default_dma_engine.dma_start(
        out=gidx_sb[:], in_=aps["gidx"].rearrange("i p s -> p i s")[:P]
    )
    ones_sb = singles.tile([P, 1], F32)
    nc.vector.memset(ones_sb[:], 1.0)

    collect = singles.tile([P, 32], F32)
    nc.vector.memset(collect[:], 0.0)

    # All assignment matmuls up-front: only the first carries a sync wait.
    ps_tiles = {}
    if stage < 2:
        for i in range(IPC):
            for h in range(H):
                ys = h * P
                s_half = hpool.tile([P, G], F32, tag="s")
                for b in range(NXB):
                    xs, xe = b * XBS, (b + 1) * XBS
                    clsb = cpool.tile([P, XBS, C], F32)
                    nc.default_dma_engine.dma_start(
                        out=clsb[:], in_=cls_v[i, ys : ys + P, xs:xe, :]
                    )
                    e = epool.tile([P, XBS, C], BF16)
                    nc.scalar.activation(e[:], clsb[:], ACTF.Exp)
                    nc.vector.tensor_reduce(
                        out=s_half[:, xs:xe], in_=e[:], axis=AXL.X, op=ALU.add
                    )
                base = (i * H + h) * 5
                nc.vector.tensor_reduce(
                    out=collect[:, base : base + 1], in_=s_half[:],
                    axis=AXL.X, op=ALU.add,
                )
        fin_ps0 = psums.tile([1, 32], F32, tag="fin")
        nc.tensor.matmul(fin_ps0[:], lhsT=ones_sb[:], rhs=collect[:], start=True, stop=True)
        final0 = singles.tile([1, 32], F32, name="final0")
        nc.vector.tensor_copy(final0[:], fin_ps0[:])
        nc.default_dma_engine.dma_start(out=aps["out"], in_=final0[:])
        return
    for i in range(IPC):
        for h in range(H):
            ps = psums.tile([P, 2 * G], F32, tag=f"ps_{i}_{h}")
            nc.tensor.matmul(
                ps[:],
                lhsT=asg_sb[:, i, h * P : h * P + P],
                rhs=asg_sb[:, i, G : 3 * G],
                start=True,
                stop=True,
            )
            ps_tiles[(i, h)] = ps

    for i in range(IPC):
        for h in range(H):
            ys, ye = h * P, (h + 1) * P
            ps = ps_tiles[(i, h)]
            # exponent extraction -> exact powers of two
            plo = tpool.tile([P, G], I32, tag="plo")
            nc.vector.tensor_scalar(
                out=plo[:], in0=ps[:, 0:G].bitcast(I32),
                scalar1=EXP_MASK, scalar2=None, op0=ALU.bitwise_and,
            )
            phi = tpool.tile([P, G], I32, tag="phi")
            nc.vector.tensor_scalar(
                out=phi[:], in0=ps[:, G : 2 * G].bitcast(I32),
                scalar1=EXP_MASK, scalar2=None, op0=ALU.bitwise_and,
            )
            q = hpool.tile([P, G], F32, tag="q")
            nc.vector.scalar_tensor_tensor(
                out=q[:], in0=phi[:].bitcast(F32), scalar=65536.0,
                in1=plo[:].bitcast(F32), op0=ALU.mult, op1=ALU.max,
            )
            qb = hpool.tile([P, G], BF16, tag="qb")
            nc.vector.tensor_copy(qb[:], q[:])
            maskt = hpool.tile([P, G], F32, tag="mask")
            nc.vector.tensor_scalar(
                out=maskt[:], in0=q[:], scalar1=1.0, scalar2=None, op0=ALU.is_ge
            )

            s_half = hpool.tile([P, G], F32, tag="s")
            sel_half = hpool.tile([P, G], F32, tag="sel")
            ab = hpool.tile([P, 4, G], F32, tag="ab")

            for b in range(NXB):
                xs, xe = b * XBS, (b + 1) * XBS
                clsb = cpool.tile([P, XBS, C], F32)
                nc.default_dma_engine.dma_start(
                    out=clsb[:], in_=cls_v[i, ys:ye, xs:xe, :]
                )
                e = epool.tile([P, XBS, C], BF16)
                nc.scalar.activation(e[:], clsb[:], ACTF.Exp)
                nc.vector.tensor_reduce(
                    out=s_half[:, xs:xe], in_=e[:], axis=AXL.X, op=ALU.add
                )
                if stage < 3:
                    continue
                proj = ppool.tile([P, NI, 1], F32, tag="proj")
                ICHUNK = 1024
                for s0 in range(0, NI, ICHUNK):
                    n_i = min(ICHUNK, NI - s0)
                    nc.gpsimd.ap_gather(
                        proj[:, s0 : s0 + n_i, :],
                        clsb[:].rearrange("p x c -> p (x c)"),
                        gidx_sb[:, i, s0 // 16 : (s0 + n_i) // 16],
                        channels=P, num_elems=XBS * C, d=1, num_idxs=n_i,
                    )
                if stage < 4:
                    nc.vector.tensor_reduce(
                        out=sel_half[:, xs:xe],
                        in_=proj[:].rearrange("p (x m) one -> p x (m one)", x=XBS),
                        axis=AXL.X, op=ALU.add,
                    )
                    continue
                o = opool.tile([P, XBS, M], F16, tag="o")
                nc.vector.tensor_tensor(
                    out=o[:],
                    in0=qb[:, xs:xe, None].broadcast_to([P, XBS, M]),
                    in1=pow2_sb[:, None, :].broadcast_to([P, XBS, M]),
                    op=ALU.is_equal,
                )
                selp = ppool.tile([P, XBS, M], F32, tag="selp")
                nc.vector.tensor_tensor(
                    out=selp[:],
                    in0=proj[:].rearrange("p (x m) one -> p x (m one)", x=XBS),
                    in1=o[:], op=ALU.mult,
                )
                nc.vector.tensor_reduce(
                    out=sel_half[:, xs:xe], in_=selp[:], axis=AXL.X, op=ALU.add
                )
                for ch in range(4):
                    abp = opool.tile([P, XBS, M], F16, tag="abp")
                    nc.vector.tensor_tensor(
                        out=abp[:],
                        in0=o[:],
                        in1=tab_sb[:, i, ch, None, :].broadcast_to([P, XBS, M]),
                        op=ALU.mult,
                    )
                    nc.vector.tensor_reduce(
                        out=ab[:, ch, xs:xe], in_=abp[:], axis=AXL.X, op=ALU.add
                    )

            # --- per-half tail ---
            if stage < 5:
                base = (i * H + h) * 5
                nc.vector.tensor_reduce(
                    out=collect[:, base : base + 1], in_=s_half[:],
                    axis=AXL.X, op=ALU.add,
                )
                nc.vector.tensor_reduce(
                    out=collect[:, base + 1 : base + 2], in_=maskt[:],
                    axis=AXL.X, op=ALU.add,
                )
                if stage >= 3:
                    nc.vector.tensor_reduce(
                        out=collect[:, base + 2 : base + 3], in_=sel_half[:],
                        axis=AXL.X, op=ALU.add,
                    )
                if stage >= 4:
                    nc.vector.tensor_reduce(
                        out=collect[:, base + 3 : base + 4], in_=ab[:, 0, :],
                        axis=AXL.X, op=ALU.add,
                    )
                continue
            pbox = hpool.tile([P, G, 4], F32, tag="pbox")
            nc.default_dma_engine.dma_start(out=pbox[:], in_=box_v[i, ys:ye, :, :])
            pobj = hpool.tile([P, G], F32, tag="pobj")
            nc.default_dma_engine.dma_start(out=pobj[:], in_=obj_v[i, ys:ye, :])

            pcx, pcy = pbox[:, :, 0], pbox[:, :, 1]
            pw, ph = pbox[:, :, 2], pbox[:, :, 3]

            def _stt(name, in0, scalar, in1, op0, op1, dt=F32):
                t = tpool.tile([P, G], dt, tag=name)
                nc.vector.scalar_tensor_tensor(
                    out=t[:], in0=in0, scalar=scalar, in1=in1, op0=op0, op1=op1
                )
                return t

            px1 = _stt("px1", pw, -0.5, pcx, ALU.mult, ALU.add)
            px2 = _stt("px2", pw, 0.5, pcx, ALU.mult, ALU.add)
            py1 = _stt("py1", ph, -0.5, pcy, ALU.mult, ALU.add)
            py2 = _stt("py2", ph, 0.5, pcy, ALU.mult, ALU.add)

            ixlo = tpool.tile([P, G], F32, tag="ixlo")
            nc.vector.tensor_tensor(ixlo[:], px1[:], ab[:, 0, :], op=ALU.max)
            ixhi = tpool.tile([P, G], F32, tag="ixhi")
            nc.vector.tensor_tensor(ixhi[:], px2[:], ab[:, 2, :], op=ALU.min)
            iylo = tpool.tile([P, G], F32, tag="iylo")
            nc.vector.tensor_tensor(iylo[:], py1[:], ab[:, 1, :], op=ALU.max)
            iyhi = tpool.tile([P, G], F32, tag="iyhi")
            nc.vector.tensor_tensor(iyhi[:], py2[:], ab[:, 3, :], op=ALU.min)

            iw = _stt("iw", ixlo[:], -1.0, ixhi[:], ALU.mult, ALU.add)
            nc.vector.tensor_scalar(iw[:], iw[:], 0.0, None, op0=ALU.max)
            ih = _stt("ih", iylo[:], -1.0, iyhi[:], ALU.mult, ALU.add)
            nc.vector.tensor_scalar(ih[:], ih[:], 0.0, None, op0=ALU.max)
            inter = tpool.tile([P, G], F32, tag="inter")
            nc.vector.tensor_tensor(inter[:], iw[:], ih[:], op=ALU.mult)

            pa = tpool.tile([P, G], F32, tag="pa")
            nc.vector.tensor_tensor(pa[:], pw, ph, op=ALU.mult)
            aw = _stt("aw", ab[:, 0, :], -1.0, ab[:, 2, :], ALU.mult, ALU.add)
            ah = _stt("ah", ab[:, 1, :], -1.0, ab[:, 3, :], ALU.mult, ALU.add)
            aa = tpool.tile([P, G], F32, tag="aa")
            nc.vector.tensor_tensor(aa[:], aw[:], ah[:], op=ALU.mult)

            un = tpool.tile([P, G], F32, tag="un")
            nc.vector.tensor_tensor(un[:], pa[:], aa[:], op=ALU.add)
            un2 = _stt("un2", inter[:], -1.0, un[:], ALU.mult, ALU.add)
            nc.vector.tensor_scalar(un2[:], un2[:], EPS, None, op0=ALU.add)
            rec = tpool.tile([P, G], F32, tag="rec")
            if stage >= 6:
                nc.vector.reciprocal_approx_fast(rec[:], un2[:])
            else:
                nc.vector.tensor_copy(rec[:], un2[:])
            iou = tpool.tile([P, G], F32, tag="iou")
            nc.vector.tensor_tensor(iou[:], inter[:], rec[:], op=ALU.mult)

            # focal
            lns = tpool.tile([P, G], F32, tag="lns")
            nc.scalar.activation(lns[:], s_half[:], ACTF.Ln)
            ce = _stt("ce", sel_half[:], -1.0, lns[:], ALU.mult, ALU.add)
            pt = tpool.tile([P, G], F32, tag="pt")
            nc.scalar.activation(pt[:], ce[:], ACTF.Exp, scale=-1.0)
            q1 = tpool.tile([P, G], F32, tag="q1")
            nc.vector.tensor_scalar(
                out=q1[:], in0=pt[:], scalar1=-1.0, scalar2=1.0,
                op0=ALU.mult, op1=ALU.add,
            )
            q2 = tpool.tile([P, G], F32, tag="q2")
            nc.vector.tensor_tensor(q2[:], q1[:], q1[:], op=ALU.mult)
            flp = tpool.tile([P, G], F32, tag="flp")
            nc.vector.tensor_tensor(flp[:], q2[:], ce[:], op=ALU.mult)

            # objectness bce
            lp = tpool.tile([P, G], F32, tag="lp")
            nc.scalar.activation(lp[:], pobj[:], ACTF.Ln)
            nc.vector.tensor_scalar(lp[:], lp[:], -100.0, None, op0=ALU.max)
            om = tpool.tile([P, G], F32, tag="om")
            nc.vector.tensor_scalar(
                out=om[:], in0=pobj[:], scalar1=-1.0, scalar2=1.0,
                op0=ALU.mult, op1=ALU.add,
            )
            l1m = tpool.tile([P, G], F32, tag="l1m")
            nc.scalar.activation(l1m[:], om[:], ACTF.Ln)
            nc.vector.tensor_scalar(l1m[:], l1m[:], -100.0, None, op0=ALU.max)
            d = tpool.tile([P, G], F32, tag="d")
            nc.vector.tensor_tensor(d[:], lp[:], l1m[:], op=ALU.subtract)

            # accumulate into collect slots
            base = (i * H + h) * 5

            # tensor_tensor_reduce faults at runtime on this stack; use
            # mult + reduce instead (ALPHA applied host-side).
            def _masked_sum(src, slot, tag="tr1"):
                tr = tpool.tile([P, G], F32, tag=tag)
                nc.vector.tensor_tensor(tr[:], src[:], maskt[:], op=ALU.mult)
                nc.vector.tensor_reduce(
                    out=collect[:, slot : slot + 1], in_=tr[:],
                    axis=AXL.X, op=ALU.add,
                )

            _masked_sum(flp, base + 0, tag="tr1")
            _masked_sum(iou, base + 1, tag="tr2")
            nc.vector.tensor_reduce(
                out=collect[:, base + 2 : base + 3], in_=maskt[:],
                axis=AXL.X, op=ALU.add,
            )
            _masked_sum(d, base + 3, tag="tr3")  # noqa: B023
            nc.vector.tensor_reduce(
                out=collect[:, base + 4 : base + 5], in_=l1m[:],
                axis=AXL.X, op=ALU.add,
            )

    # cross-partition sum via PE: ones^T @ collect -> [1, 32]
    fin_ps = psums.tile([1, 32], F32)
    nc.tensor.matmul(fin_ps[:], lhsT=ones_sb[:], rhs=collect[:], start=True, stop=True)
    final = singles.tile([1, 32], F32)
    nc.vector.tensor_copy(final[:], fin_ps[:])
    nc.default_dma_engine.dma_start(out=aps["out"], in_=final[:])


def build_program(G=256, C=80, M=32, IPC=2, XBS=64, stage=99):
    N = G * G
    NI = XBS * M
    nc = bacc.Bacc("TRN2", target_bir_lowering=False, debug=False)
    aps = {
        "cls": nc.dram_tensor("cls", [IPC, N, C], F32, kind="ExternalInput").ap(),
        "boxes": nc.dram_tensor("boxes", [IPC, N, 4], F32, kind="ExternalInput").ap(),
        "obj": nc.dram_tensor("obj", [IPC, N], F32, kind="ExternalInput").ap(),
        "asg": nc.dram_tensor("asg", [IPC, M, 3 * G], F32, kind="ExternalInput").ap(),
        "pow2": nc.dram_tensor("pow2", [128, M], BF16, kind="ExternalInput").ap(),
        "tab16": nc.dram_tensor("tab16", [IPC, 128, 4, M], F16, kind="ExternalInput").ap(),
        "gidx": nc.dram_tensor("gidx", [IPC, 128, NI // 16], I16, kind="ExternalInput").ap(),
        "out": nc.dram_tensor("out", [1, 32], F32, kind="ExternalOutput").ap(),
    }
    with tile.TileContext(nc) as tc, ExitStack() as ctx:
        _kernel_body(ctx, tc, aps, G, C, M, IPC, XBS, stage=stage)
    nc.compile()
    return nc


def make_core_inputs(classifications, boxes, objectness, gt_boxes, gt_labels,
                     num_objects, G, C, M, IPC, XBS, img_lo):
    """Host prep for one core handling images [img_lo, img_lo+IPC)."""
    NI = XBS * M
    # matches jnp.linspace(0.0, 1.0, G) bitwise: iota * fp32(1/(G-1))
    coords = np.arange(G, dtype=np.float32) * np.float32(1.0 / (G - 1))
    asg = np.zeros((IPC, M, 3 * G), np.float32)
    lhsT = asg[:, :, 0:G]
    rhsm = asg[:, :, G : 3 * G]
    tab16 = np.zeros((IPC, 128, 4, M), np.float16)
    gidx = np.zeros((IPC, 128, NI // 16), np.int16)
    for ii in range(IPC):
        img = img_lo + ii
        K = int(num_objects[img])
        gb = gt_boxes[img].astype(np.float32)
        x1 = gb[:, 0] - gb[:, 2] * np.float32(0.5)
        x2 = gb[:, 0] + gb[:, 2] * np.float32(0.5)
        y1 = gb[:, 1] - gb[:, 3] * np.float32(0.5)
        y2 = gb[:, 1] + gb[:, 3] * np.float32(0.5)
        for j in range(K):
            row = ((coords >= y1[j]) & (coords <= y2[j])).astype(np.float32)
            col = ((coords >= x1[j]) & (coords <= x2[j])).astype(np.float32)
            lhsT[ii, j, :] = row * np.float32(2.0 ** (j % 16))
            if j < 16:
                rhsm[ii, j, 0:G] = col
            else:
                rhsm[ii, j, G : 2 * G] = col
            tab16[ii, :, 0, j] = np.float16(x1[j])
            tab16[ii, :, 1, j] = np.float16(y1[j])
            tab16[ii, :, 2, j] = np.float16(x2[j])
            tab16[ii, :, 3, j] = np.float16(y2[j])
        labels = gt_labels[img].astype(np.int64)
        idx_vals = np.zeros(NI, np.int64)
        for i2 in range(NI):
            x, j = divmod(i2, M)
            cj = labels[j] if j < K else 0
            idx_vals[i2] = x * C + cj
        for p in range(128):
            for s in range(NI // 16):
                gidx[ii, p, s] = idx_vals[s * 16 + (p % 16)]
    pow2 = np.zeros((128, M), np.float32)
    pow2[:, :] = (2.0 ** np.arange(M, dtype=np.float64)).astype(np.float32)
    import ml_dtypes
    pow2 = pow2.astype(ml_dtypes.bfloat16)
    sl = slice(img_lo, img_lo + IPC)
    return {
        "cls": np.ascontiguousarray(classifications[sl]),
        "boxes": np.ascontiguousarray(boxes[sl]),
        "obj": np.ascontiguousarray(objectness[sl]),
        "asg": asg,
        "pow2": pow2,
        "tab16": tab16,
        "gidx": gidx,
    }


def combine_outputs(outs, num_objects, B, N, IPC, H):
    """outs: list per core of [1, 32] arrays. Returns scalar np.float32."""
    cls_sums = np.zeros(B, np.float64)
    box_raw = np.zeros(B, np.float64)
    counts = np.zeros(B, np.float64)
    obj_acc = 0.0
    for c, o in enumerate(outs):
        o = o.reshape(-1).astype(np.float64)
        for ii in range(IPC):
            img = c * IPC + ii
            for h in range(H):
                base = (ii * H + h) * 5
                cls_sums[img] += ALPHA * o[base + 0]
                box_raw[img] += o[base + 1]
                counts[img] += o[base + 2]
                obj_acc += o[base + 3] + o[base + 4]
    box_sums = counts - box_raw  # sum mask*(1-iou)
    cls_per = np.where(counts > 0, cls_sums / np.maximum(counts, 1.0), 0.0)
    box_per = np.where(counts > 0, box_sums / np.maximum(counts, 1.0), 0.0)
    cls_sum = cls_per.sum()
    box_sum = box_per.sum()
    total_pos = counts.sum()
    cls_loss = cls_sum / B if total_pos > 0 else cls_sum
    box_loss = box_sum / B if total_pos > 0 else box_sum
    obj_loss = -obj_acc / (B * N)
    total = W_CLS * cls_loss + W_BOX * box_loss + W_OBJ * obj_loss
    return np.array(total, dtype=np.float32)


_PROGRAM_CACHE = {}


def kernel(classifications, boxes, objectness, gt_boxes, gt_labels, num_objects):
    B, N, C = classifications.shape
    M = gt_boxes.shape[1]
    G = int(round(N ** 0.5))
    NCORES = 8
    IPC = B // NCORES
    XBS = 64
    P = min(G, 128)
    H = G // P

    key = (G, C, M, IPC, XBS)
    if key not in _PROGRAM_CACHE:
        _PROGRAM_CACHE[key] = build_program(G, C, M, IPC, XBS)
    nc = _PROGRAM_CACHE[key]

    in_maps = [
        make_core_inputs(
            np.asarray(classifications), np.asarray(boxes), np.asarray(objectness),
            np.asarray(gt_boxes), np.asarray(gt_labels), np.asarray(num_objects),
            G, C, M, IPC, XBS, c * IPC,
        )
        for c in range(NCORES)
    ]
    res = run_bass_kernel_spmd(nc, in_maps, list(range(NCORES)))
    outs = [r["out"] for r in res.results]
    return combine_outputs(outs, np.asarray(num_objects), B, N, IPC, H)
